# revision 9
# baseline (speedup 1.0000x reference)
"""LIF spike kernel (T=4 scan with threshold reset) on 8 TRN2 NeuronCores.

Recurrence per element (tau=1, thresh=1):
    s_t     = m_{t-1} + x_t
    spike_t = (s_t > 1)           -> output
    m_t     = s_t * (s_t <= 1)    -> threshold reset

Sharding: pure data-parallel over the batch axis (dim 1, 64 -> 8 per core).

v3 design. The old kernel emitted f32 spikes (16 MiB/core out) and ran all
compares on the Vector engine; both DMA and DVE sat ~90+ us busy. Here:
  - spikes leave the device as int8 sign planes: q_t = Sign(1 - s_t) in
    {-1,0,1}, computed on the otherwise-idle Scalar (ACT) engine straight
    from the s-slices (exact at the threshold: Sign is not interpolated,
    and s==1 maps to q==0 -> no spike, matching the strict >). Output
    traffic drops 4x (16 MiB -> 4 MiB); host maps q==-1 -> 1.0f (free).
  - DVE does only the serial recurrence: 3 tensor_tensor adds + 3
    scalar_tensor_tensor threshold resets per chunk, s_t in place over the
    x-slices of one coalesced [128, 4F] tile, so ACT reads never block the
    DVE chain (resets write a separate M tile).
  - one coalesced HWDGE load per chunk; one int8 store per (t, chunk).
Engine busy ~= DMA 21 MB ~61 us, DVE ~57 us, ACT ~31 us.
"""

import numpy as np

import concourse.bacc as bacc
import concourse.mybir as mybir
import concourse.tile as tile
from concourse import bass_utils

T = 4
B_FULL = 64
C, H, W = 128, 32, 32
N_CORES = 8
B_LOC = B_FULL // N_CORES            # 8
N = B_LOC * C * H * W                # 1048576 elements per core per timestep
P = 128                              # SBUF partitions

_LE = mybir.AluOpType.is_le
_MUL = mybir.AluOpType.mult
_ADD = mybir.AluOpType.add
_F32 = mybir.dt.float32
_I8 = mybir.dt.int8
_SIGN = mybir.ActivationFunctionType.Sign

F = 2048                 # free dim per chunk; nchunk = N/(P*F)
_nc_cache = None


def _build(F=F, bufs=3):
    nchunk = N // (P * F)
    nc = bacc.Bacc(
        "TRN2",
        target_bir_lowering=False,
        debug=False,
        enable_asserts=False,
    )
    x_d = nc.dram_tensor("x", [T, N], _F32, kind="ExternalInput").ap()
    y_d = nc.dram_tensor("y", [nchunk, P, T, F], _I8, kind="ExternalOutput").ap()
    # [n, p, t, f]: per (chunk, partition) the 4 timesteps' runs
    xc = x_d.rearrange("t (n p f) -> n p t f", p=P, f=F)

    with tile.TileContext(nc) as tc:
        with (
            tc.tile_pool(name="xx", bufs=bufs) as xp,
            tc.tile_pool(name="mm", bufs=2) as mp,
            tc.tile_pool(name="qq", bufs=bufs) as qp,
        ):
            for j in range(nchunk):
                # per-plane tiles/loads: DVE's first op only waits on the
                # 1 MB t=0 plane, not a whole 4 MB chunk (HWDGE is FIFO,
                # so plane (j,0) completes first)
                sl = []
                for t in range(T):
                    xt = xp.tile([P, F], _F32, tag=f"x{t}", name=f"x{t}_{j}")
                    nc.sync.dma_start(xt[:], xc[j, :, t])
                    sl.append(xt[:])
                m = mp.tile([P, F], _F32, tag="m", name=f"m_{j}")

                v = nc.vector
                # DVE recurrence; ACT signs trail behind reading the s
                # planes (s_t in place over x_t); stores ride the ACT ring
                # right after each sign, keeping the sync ring loads-only
                v.scalar_tensor_tensor(m[:], sl[0], 1.0, sl[0], _LE, _MUL)
                for t in range(T):
                    if t > 0:
                        if t == T - 1 and j < nchunk - 1:
                            # terminal add: no reset follows, so GPSIMD can
                            # take it without re-entering the DVE chain
                            # (last chunk stays on DVE to keep the tail short)
                            nc.gpsimd.tensor_tensor(sl[t], m[:], sl[t], _ADD)
                        else:
                            v.tensor_tensor(sl[t], m[:], sl[t], _ADD)
                        if t < T - 1:
                            v.scalar_tensor_tensor(
                                m[:], sl[t], 1.0, sl[t], _LE, _MUL
                            )
                    q = qp.tile([P, F], _I8, tag=f"q{t}", name=f"q{t}_{j}")
                    nc.scalar.activation(
                        q[:], sl[t], _SIGN, bias=1.0, scale=-1.0
                    )
                    nc.scalar.dma_start(y_d[j, :, t], q[:])

    nc.compile()
    return nc


def _get_nc():
    global _nc_cache
    if _nc_cache is None:
        _nc_cache = _build()
    return _nc_cache


def _run(x, **spmd_kwargs):
    x = np.asarray(x, dtype=np.float32)
    assert x.shape == (T, B_FULL, C, H, W), x.shape
    in_maps = [
        {
            "x": np.ascontiguousarray(
                x[:, c * B_LOC : (c + 1) * B_LOC]
            ).reshape(T, N)
        }
        for c in range(N_CORES)
    ]
    res = bass_utils.run_bass_kernel_spmd(
        _get_nc(), in_maps, core_ids=list(range(N_CORES)), **spmd_kwargs
    )
    nchunk = N // (P * F)
    out = np.empty((T, B_FULL, C, H, W), dtype=np.float32)
    for c in range(N_CORES):
        y = res.results[c]["y"]
        sp = (
            y.reshape(nchunk, P, T, F).transpose(2, 0, 1, 3).reshape(T, N)
            == -1
        ).astype(np.float32)
        out[:, c * B_LOC : (c + 1) * B_LOC] = sp.reshape(T, B_LOC, C, H, W)
    return out, res


def kernel(x):
    out, _ = _run(x)
    return out
